# revision 1
# baseline (speedup 1.0000x reference)
"""Causal single-head attention (B=4, S=2048, D=1024) on 8 Trainium2 cores.

Sharding: core c -> (batch b = c//2, half h = c%2). Each core owns 1024
query rows: the 8 seq-blocks of 128 rows with global block index
g = 2*j + h (j = 0..7), which balances causal work between the two
halves. Each core projects K/V for all 2048 keys of its batch and Q for
its own rows, runs block-causal attention, and writes out^T [D, 1024].
The host transposes/scatters the per-core outputs back together.

On-chip layout: everything contracts over the partition dim.
  Q^T[d, q]  = sum_e WqT[e, d] * xT[e, q]   (lhsT = WqT chunk, rhs = xT)
  K^T[d, s]  likewise
  V[s, d]    = sum_e xT[e, s] * WvT[e, d]   (lhsT = xT chunk, rhs = WvT)
  S^T[k, q]  = sum_d KT[d, k] * QT[d, q]    (scores, transposed)
  E^T[k, q]  = mask * exp(S^T)              (ACT exp, DVE mask-mult)
  den[q]     = sum_k E^T[k, q]              (ones-matmul on PE)
  O^T[d, q]  = sum_k V[k, d] * E^T[k, q] / den[q]

Each core projects Q/K/V only for its own 1024 rows; K^T and V are
exchanged with the pair partner via a 2-core AllGather (DRAM bounce),
giving every core all 2048 keys in "gathered" order (even half then odd
half) -- identical on both cores, so the SPMD program is uniform and
causality is pure mask data computed on the host per core.
"""

import sys

sys.path.insert(0, "/opt/trn_rl_repo")

import numpy as np
import ml_dtypes

import concourse.bass as bass
import concourse.tile as tile
import concourse.mybir as mybir
from concourse.bass_utils import run_bass_kernel_spmd
from concourse.vector_clock import ScopedClock, VectorClock

BF16 = mybir.dt.bfloat16
F32 = mybir.dt.float32
AF = mybir.ActivationFunctionType

B, S, D = 4, 2048, 1024
SL = 1024  # local query rows per core
N_CORES = 8
NEG = 0.0  # masks are multiplicative 0/1


def _patch_ldw_opt():
    """Re-enable walrus's redundant-LDWEIGHTS elimination (off by default in
    this runtime). Our matmuls come in same-stationary pairs, so eliding the
    duplicate weight load saves about half the LDWEIGHTS traffic."""
    import os
    import concourse.bass_utils as bu

    if getattr(bu, "_ldw_opt_patched", False):
        return
    bu._ldw_opt_patched = True
    orig = bu.run_command

    def wrapped(argv, **kwargs):
        if os.environ.get("KERNEL_LDW_OPT") == "1":
            argv = [
                a.replace("--enable-ldw-opt=false", "--enable-ldw-opt=true")
                for a in argv
            ]
        return orig(argv, **kwargs)

    bu.run_command = wrapped


def _patch_tile_runtime():
    """Two local-runtime fixes:
    1. This walrus build rejects >1 semaphore wait on a Drain instruction;
       split the TileContext exit drain into one drain per awaited proc.
    2. Raise the stale 192KB/partition SBUF cap to 208KB (cayman usable).
    """
    import concourse.tile_utils as tile_utils

    tile_utils.max_sbuf_usage = 208 * 1024

    def _drain_and_barrier(self, tick_clock, wait_clock):
        g = tick_clock.global_clock
        n = len(g)
        for j in range(n):
            if g[j] > 0:
                sub = VectorClock([g[i] if i == j else 0 for i in range(n)])
                drain_inst = self.nc.sync.drain()
                wait_clock.add_sem_waits(drain_inst.ins, ScopedClock({None: sub}))
        self.nc.all_engine_barrier()
        assert self.sems is not None
        popped = self.nc._tile_sem_poison_stack.pop()
        assert popped is self._sem_poison
        self.nc.clear_and_free_semaphores(list(self.sems.allocated().values()))
        self.nc.all_engine_barrier()

    tile.TileContext._drain_and_barrier = _drain_and_barrier


def _local_blocks(h):
    """Global 128-row block indices owned by half h, in local order."""
    return [2 * j + h for j in range(8)]


def _key_perm_blocks(h=None):
    """Key 128-blocks in on-chip (gathered) order: even half then odd half.
    Identical on both cores of a pair (AllGather slot order)."""
    return _local_blocks(0) + _local_blocks(1)


def _chunk_list(t):
    """Key-chunk slots (into the permuted order) computed for q-tile t."""
    return list(range(0, 4 * t + 4)) + list(range(8, 12 + 4 * t))


def _masked_chunks(t):
    """(slot, mask_j) pairs needing a mask for q-tile t: own and partner
    chunks 4t..4t+3 (mask_j 0..3 own, 4..7 partner)."""
    out = []
    for j in range(4):
        out.append((4 * t + j, j))  # own chunk 4t+j
        out.append((8 + 4 * t + j, 4 + j))  # partner chunk 4t+j
    return out


def build_masks(h):
    """mask[t, j, k, q] in {0,1}: for q-tile t, mask_j as in _masked_chunks."""
    qg = np.empty(SL, dtype=np.int64)  # local q index -> global row
    for j in range(8):
        qg[j * 128 : (j + 1) * 128] = 128 * (2 * j + h) + np.arange(128)
    kperm = _key_perm_blocks(h)
    mask = np.zeros((2, 8, 128, 512), dtype=np.float32)
    for t in range(2):
        qs = qg[t * 512 : (t + 1) * 512]  # [512] global q rows
        for slot, j in _masked_chunks(t):
            kb = kperm[slot]  # global key block
            ks = 128 * kb + np.arange(128)  # [128] global key rows
            mask[t, j] = (qs[None, :] >= ks[:, None]).astype(np.float32)
    return mask.astype(ml_dtypes.bfloat16)


def _split_waits(nc, limit=1, dma_limit=1):
    """This walrus build accepts at most `limit` semaphore waits per
    engine instruction (and `dma_limit` per DMA descriptor). Hoist extra
    waits onto same-engine NoOps inserted immediately before the
    offending instruction (same program point, so ordering semantics are
    unchanged; for DMAs the wait moves from trigger-time to issue-time on
    the issuing sequencer, which only strengthens ordering)."""
    n_split = 0
    for f in nc.m.functions:
        for bb in f.blocks:
            new = []
            for inst in bb.instructions:
                si = inst.sync_info
                waits = list(si.on_wait) if si and si.on_wait else []
                lim = dma_limit if type(inst).__name__ == "InstDMACopy" else limit
                if len(waits) > lim:
                    extra, keep = waits[:-lim], waits[-lim:]
                    for j in range(0, len(extra), limit):
                        nop = mybir.InstNoOp(
                            name=f"{inst.name}-wsplit{j}", ins=[], outs=[]
                        )
                        nop.engine = inst.engine
                        nop.sync_info = mybir.SyncInfo(
                            on_wait=extra[j : j + limit], on_update=[]
                        )
                        new.append(nop)
                        n_split += 1
                    si.on_wait = keep
                new.append(inst)
            bb.instructions[:] = new
    return n_split


def _dedup_ldw(nc):
    """With the PE stream pinned to emission order, paired matmuls share
    their stationary operand back-to-back; convert the second (identical)
    InstLdweights into a NoOp carrying the same sync_info."""
    n = 0
    for f in nc.m.functions:
        for bb in f.blocks:
            prev_sig = None
            for idx, inst in enumerate(bb.instructions):
                if inst.engine != mybir.EngineType.PE:
                    continue
                nm = type(inst).__name__
                if nm == "InstLdweights":
                    sig = str(inst.ins[0])
                    if sig == prev_sig:
                        nop = mybir.InstNoOp(
                            name=f"{inst.name}-ldwdup", ins=[], outs=[]
                        )
                        nop.engine = inst.engine
                        nop.sync_info = inst.sync_info
                        bb.instructions[idx] = nop
                        n += 1
                    prev_sig = sig
                elif nm != "InstMatmult":
                    prev_sig = None
    return n


def build_program(split_waits=True, reps=1, mock_cc=False, pin_pe=True):
    _patch_tile_runtime()
    _patch_ldw_opt()
    nc = bass.Bass("TRN2", target_bir_lowering=False, debug=False)

    xTp = nc.dram_tensor("xTp", [D, SL], BF16, kind="ExternalInput").ap()
    wqT = nc.dram_tensor("wqT", [D, D], BF16, kind="ExternalInput").ap()
    wkT = nc.dram_tensor("wkT", [D, D], BF16, kind="ExternalInput").ap()
    wvT = nc.dram_tensor("wvT", [D, D], BF16, kind="ExternalInput").ap()
    bq32 = nc.dram_tensor("bq32", [D], F32, kind="ExternalInput").ap()
    bk = nc.dram_tensor("bk", [D], F32, kind="ExternalInput").ap()
    bvb = nc.dram_tensor("bvb", [128, D], BF16, kind="ExternalInput").ap()
    maskd = nc.dram_tensor("mask", [2, 8, 128, 512], BF16, kind="ExternalInput").ap()
    outT = nc.dram_tensor("outT", [D, SL], F32, kind="ExternalOutput").ap()

    with tile.TileContext(nc) as tc:
        with (
            tc.tile_pool(name="persist", bufs=1) as persist,
            tc.tile_pool(name="wpool", bufs=2) as wpool,
            tc.tile_pool(name="xpool", bufs=2) as xpool,
            tc.tile_pool(name="xg", bufs=1) as xg,
            tc.tile_pool(name="mk", bufs=8) as mkp,
            tc.tile_pool(name="scratch", bufs=4) as scratch,
            tc.tile_pool(name="outp", bufs=6) as outp,
            tc.tile_pool(name="bc", bufs=2) as bcp,
            tc.tile_pool(name="dram", bufs=1, space="DRAM") as drp,
            tc.tile_pool(name="ps", bufs=8, space="PSUM") as psp,
        ):
            def emit_body():
                # Pin the PE stream to emission order so same-stationary
                # matmul pairs stay adjacent (enables the LDWEIGHTS dedup
                # post-pass); deps are order-only on a single engine.
                last_mm = [None]

                def MM(*a, **kw):
                    inst = nc.tensor.matmul(*a, **kw)
                    if pin_pe:
                        if last_mm[0] is not None:
                            bass._add_dep_helper(
                                inst.ins, last_mm[0].ins, False, "pe-order-pin"
                            )
                        last_mm[0] = inst
                    return inst

                # ---- persistent SBUF tiles ----
                QT = persist.tile([128, 8, SL], BF16)  # [dp, dc, q]
                KT = persist.tile([128, 8, S], BF16)  # [dp, dc, k]
                V = persist.tile([128, 16, D], BF16)  # [kp, kc, d]
                ET0 = persist.tile([128, 8, 512], BF16)  # q-tile 0
                ET1 = persist.tile([128, 16, 512], BF16)  # q-tile 1
                ET = [ET0, ET1]
                bq_t = persist.tile([128, 8], F32)
                bk_t = persist.tile([128, 8], F32)
                bv_t = persist.tile([128, D], BF16)
                ones = persist.tile([128, 1], BF16)
                ones_row = persist.tile([1, 128], F32)

                nc.vector.memset(ones, 1.0)
                nc.vector.memset(ones_row, 1.0)
                nc.sync.dma_start(out=bq_t, in_=bq32.rearrange("(c p) -> p c", p=128))
                nc.sync.dma_start(out=bk_t, in_=bk.rearrange("(c p) -> p c", p=128))
                nc.sync.dma_start(out=bv_t, in_=bvb)

                # ---- projections (own 1024 rows only; K^T and V are
                # exchanged with the pair partner via AllGather). Each
                # weight-chunk LDWEIGHTS feeds two matmuls.
                def load_w(dram, split=False):
                    wt = wpool.tile([128, 8, D], BF16, tag="w")
                    src_ap = dram.rearrange("(c p) d -> p c d", p=128)
                    if split:
                        # per-d-chunk DMAs: the first matmul group only
                        # waits on its own 256KB slice, not the whole 2MB
                        for dc in range(8):
                            nc.sync.dma_start(
                                out=wt[:, :, dc * 128 : (dc + 1) * 128],
                                in_=src_ap[:, :, dc * 128 : (dc + 1) * 128],
                            )
                    else:
                        nc.sync.dma_start(out=wt, in_=src_ap)
                    return wt

                def load_xt(sh):
                    xt = xpool.tile([128, 8, 512], BF16, tag="xt")
                    nc.sync.dma_start(
                        out=xt,
                        in_=xTp[:, sh * 512 : (sh + 1) * 512].rearrange(
                            "(c p) s -> p c s", p=128
                        ),
                    )
                    return xt

                pair_groups = [[0, 1], [2, 3], [4, 5], [6, 7]]

                # Phase K: K^T for own keys, then kick the exchange
                wk_t = load_w(wkT, split=True)
                xta, xtb = load_xt(0), load_xt(1)
                KTown = xg.tile([128, 8, SL], BF16, tag="xg", name="KTown")
                for dc in range(8):
                    ps0 = psp.tile([128, 512], F32, tag="ps")
                    ps1 = psp.tile([128, 512], F32, tag="ps")
                    for ec in range(8):
                        w = wk_t[:, ec, dc * 128 : (dc + 1) * 128]
                        MM(ps0, w, xta[:, ec, :], start=(ec == 0), stop=(ec == 7))
                        MM(ps1, w, xtb[:, ec, :], start=(ec == 0), stop=(ec == 7))
                    for sh, ps in ((0, ps0), (1, ps1)):
                        nc.scalar.activation(
                            out=KTown[:, dc, sh * 512 : (sh + 1) * 512],
                            in_=ps,
                            func=AF.Identity,
                            bias=bk_t[:, dc : dc + 1],
                            scale=1.0,
                        )
                ibK = drp.tile([128, 8, SL], BF16, tag="ibK", name="ibK")
                obK = drp.tile([2, 128, 8, SL], BF16, tag="obK", name="obK")
                if mock_cc:
                    nc.sync.dma_start(out=ibK, in_=KTown)
                    for g in range(2):
                        nc.sync.dma_start(out=obK[g], in_=ibK)
                else:
                    nc.gpsimd.dma_start(out=ibK, in_=KTown)
                    nc.gpsimd.collective_compute(
                        "AllGather",
                        mybir.AluOpType.bypass,
                        replica_groups=pair_groups,
                        ins=[ibK[:]],
                        outs=[obK[:]],
                    )
                for g in range(2):
                    nc.sync.dma_start(
                        out=KT[:, :, g * SL : (g + 1) * SL], in_=obK[g]
                    )

                # Phase Q
                wq_t = load_w(wqT)
                for dc in range(8):
                    ps0 = psp.tile([128, 512], F32, tag="ps")
                    ps1 = psp.tile([128, 512], F32, tag="ps")
                    for ec in range(8):
                        w = wq_t[:, ec, dc * 128 : (dc + 1) * 128]
                        MM(ps0, w, xta[:, ec, :], start=(ec == 0), stop=(ec == 7))
                        MM(ps1, w, xtb[:, ec, :], start=(ec == 0), stop=(ec == 7))
                    for sh, ps in ((0, ps0), (1, ps1)):
                        nc.scalar.activation(
                            out=QT[:, dc, sh * 512 : (sh + 1) * 512],
                            in_=ps,
                            func=AF.Identity,
                            bias=bq_t[:, dc : dc + 1],
                            scale=1.0 / 32.0,
                        )

                # Phase V: own keys; x^T chunk stationary feeds both d-halves
                wv_t = load_w(wvT)
                Vown = xg.tile([128, 8, D], BF16, tag="xg", name="Vown")
                for kc in range(8):
                    xt = xta if kc < 4 else xtb
                    kl = kc % 4
                    ps0 = psp.tile([128, 512], F32, tag="ps")
                    ps1 = psp.tile([128, 512], F32, tag="ps")
                    for ec in range(8):
                        xl = xt[:, ec, kl * 128 : (kl + 1) * 128]
                        MM(ps0, xl, wv_t[:, ec, 0:512], start=(ec == 0), stop=(ec == 7))
                        MM(ps1, xl, wv_t[:, ec, 512:1024], start=(ec == 0), stop=(ec == 7))
                    for dh, ps in ((0, ps0), (1, ps1)):
                        nc.vector.tensor_tensor(
                            out=Vown[:, kc, dh * 512 : (dh + 1) * 512],
                            in0=ps,
                            in1=bv_t[:, dh * 512 : (dh + 1) * 512],
                            op=mybir.AluOpType.add,
                        )
                ibV = drp.tile([128, 8, D], BF16, tag="ibV", name="ibV")
                obV = drp.tile([2, 128, 8, D], BF16, tag="obV", name="obV")
                if mock_cc:
                    nc.sync.dma_start(out=ibV, in_=Vown)
                    for g in range(2):
                        nc.sync.dma_start(out=obV[g], in_=ibV)
                else:
                    nc.gpsimd.dma_start(out=ibV, in_=Vown)
                    nc.gpsimd.collective_compute(
                        "AllGather",
                        mybir.AluOpType.bypass,
                        replica_groups=pair_groups,
                        ins=[ibV[:]],
                        outs=[obV[:]],
                    )
                for g in range(2):
                    nc.sync.dma_start(out=V[:, 8 * g : 8 * g + 8, :], in_=obV[g])

                # ---- attention scores: both q-tiles per key chunk so each
                # K^T chunk LDWEIGHTS feeds two matmuls (q-tile 0's chunk set
                # is a subset of q-tile 1's)
                set0 = set(_chunk_list(0))
                c1 = _chunk_list(1)
                md0 = dict(_masked_chunks(0))
                md1 = dict(_masked_chunks(1))
                n0 = len(_chunk_list(0))
                dp0 = psp.tile([1, 512], F32, tag="ps")
                dp1 = psp.tile([1, 512], F32, tag="ps")

                def do_exp(t, i, c, sp, md):
                    if c in md:
                        mt = mkp.tile([128, 512], BF16, tag="mk")
                        nc.sync.dma_start(out=mt, in_=maskd[t, md[c]])
                        ex = scratch.tile([128, 512], BF16, tag="ex")
                        nc.scalar.activation(out=ex, in_=sp, func=AF.Exp)
                        nc.vector.tensor_tensor(
                            out=ET[t][:, i, :],
                            in0=ex,
                            in1=mt,
                            op=mybir.AluOpType.mult,
                        )
                    else:
                        nc.scalar.activation(out=ET[t][:, i, :], in_=sp, func=AF.Exp)

                i0 = 0
                for i1, c in enumerate(c1):
                    sp1 = psp.tile([128, 512], F32, tag="ps")
                    sp0 = psp.tile([128, 512], F32, tag="ps", name=f"sp0_{i1}") if c in set0 else None
                    for dc in range(8):
                        kt = KT[:, dc, c * 128 : (c + 1) * 128]
                        MM(sp1, kt, QT[:, dc, 512:1024], start=(dc == 0), stop=(dc == 7))
                        if sp0 is not None:
                            MM(sp0, kt, QT[:, dc, 0:512], start=(dc == 0), stop=(dc == 7))
                    do_exp(1, i1, c, sp1, md1)
                    if sp0 is not None:
                        do_exp(0, i0, c, sp0, md0)
                        i0 += 1

                # denominators: ones-column stationary (1-column LDW ~ free)
                for t, dp, n in ((0, dp0, n0), (1, dp1, len(c1))):
                    for i in range(n):
                        MM(
                            dp, ones, ET[t][:, i, :], start=(i == 0), stop=(i == n - 1)
                        )

                # reciprocal + partition-broadcast via rank-1 PE outer product
                rbs = []
                for dp in (dp0, dp1):
                    rec = scratch.tile([1, 512], F32, tag="rec")
                    nc.vector.reciprocal(out=rec, in_=dp)
                    rbp = psp.tile([128, 512], F32, tag="ps")
                    MM(rbp, ones_row, rec, start=True, stop=True)
                    rb = bcp.tile([128, 512], F32, tag="rb")
                    nc.vector.tensor_copy(rb, rbp)
                    rbs.append(rb)

                # ---- AV: both q-tiles per (d-half, key chunk); V chunk
                # stationary feeds two matmuls; 8 PSUM accumulators live
                i0_of = {}
                i0 = 0
                for c in c1:
                    if c in set0:
                        i0_of[c] = i0
                        i0 += 1
                last_sh = max(i for i, c in enumerate(c1) if c in set0)
                for dh in range(2):
                    av1 = [psp.tile([128, 512], F32, tag="ps", name=f"av1_{dh}_{_j}") for _j in range(4)]
                    av0 = [psp.tile([128, 512], F32, tag="ps", name=f"av0_{dh}_{_j}") for _j in range(4)]
                    for i1, c in enumerate(c1):
                        for j in range(4):
                            dc = 4 * dh + j
                            vt = V[:, c, dc * 128 : (dc + 1) * 128]
                            MM(
                                av1[j], vt, ET1[:, i1, :],
                                start=(i1 == 0), stop=(i1 == len(c1) - 1),
                            )
                            if c in set0:
                                MM(
                                    av0[j], vt, ET0[:, i0_of[c], :],
                                    start=(i0_of[c] == 0), stop=(i1 == last_sh),
                                )
                    for t, avs in ((1, av1), (0, av0)):
                        for j in range(4):
                            dc = 4 * dh + j
                            ot = outp.tile([128, 512], F32, tag="ot")
                            nc.vector.tensor_tensor(
                                out=ot, in0=avs[j], in1=rbs[t], op=mybir.AluOpType.mult
                            )
                            nc.sync.dma_start(
                                out=outT[dc * 128 : (dc + 1) * 128, t * 512 : (t + 1) * 512],
                                in_=ot,
                            )

            if reps == 1:
                emit_body()
            else:
                with tc.For_i(0, reps, 1):
                    emit_body()

    if pin_pe:
        _dedup_ldw(nc)
    if split_waits:
        _split_waits(nc)
    return nc


_prog_cache = {}


def build_in_maps(x, Wq, bq, Wk, bk, Wv, bv):
    x = np.asarray(x, dtype=np.float32)
    Wq = np.asarray(Wq, dtype=np.float32)
    Wk = np.asarray(Wk, dtype=np.float32)
    Wv = np.asarray(Wv, dtype=np.float32)
    bq = np.asarray(bq, dtype=np.float32)
    bk_np = np.asarray(bk, dtype=np.float32)
    bv_np = np.asarray(bv, dtype=np.float32)

    wq_b = np.ascontiguousarray(Wq.T).astype(ml_dtypes.bfloat16)
    wk_b = np.ascontiguousarray(Wk.T).astype(ml_dtypes.bfloat16)
    wv_b = np.ascontiguousarray(Wv.T).astype(ml_dtypes.bfloat16)
    bq32 = (bq / np.sqrt(np.float32(D))).astype(np.float32)
    bvb = np.broadcast_to(bv_np.astype(ml_dtypes.bfloat16), (128, D)).copy()
    masks = [build_masks(h) for h in range(2)]

    in_maps = []
    for c in range(N_CORES):
        b, h = divmod(c, 2)
        own = np.concatenate(
            [128 * g + np.arange(128) for g in _local_blocks(h)]
        )
        xTp = np.ascontiguousarray(x[b].T[:, own]).astype(ml_dtypes.bfloat16)
        in_maps.append(
            {
                "xTp": xTp,
                "wqT": wq_b,
                "wkT": wk_b,
                "wvT": wv_b,
                "bq32": bq32,
                "bk": bk_np,
                "bvb": bvb,
                "mask": masks[h],
            }
        )
    return in_maps


def unshard(outTs):
    """outTs: list of 8 per-core outT arrays [D, SL] -> full [B, S, D]."""
    out = np.empty((B, S, D), dtype=np.float32)
    for c in range(N_CORES):
        b, h = divmod(c, 2)
        rows = np.concatenate([128 * g + np.arange(128) for g in _local_blocks(h)])
        out[b, rows, :] = outTs[c].T
    return out


def kernel(x, Wq, bq, Wk, bk, Wv, bv):
    if "nc" not in _prog_cache:
        _prog_cache["nc"] = build_program()
    nc = _prog_cache["nc"]
    in_maps = build_in_maps(x, Wq, bq, Wk, bk, Wv, bv)
    try:
        res = run_bass_kernel_spmd(nc, in_maps, list(range(N_CORES)))
    except Exception:
        # one retry absorbs transient device/mesh hiccups
        import time as _t

        _t.sleep(2.0)
        res = run_bass_kernel_spmd(nc, in_maps, list(range(N_CORES)))
    return unshard([res.results[c]["outT"] for c in range(N_CORES)])


# ---------------------------------------------------------------------------
# Timing: run a reps-times-looped program variant through a no-donation jit
# wrapper with device-resident inputs, and difference wall-clock floors.


def _make_runner(nc, n_cores=N_CORES):
    import jax
    from jax.experimental.shard_map import shard_map
    from jax.sharding import Mesh, PartitionSpec
    from concourse import bass2jax

    bass2jax.install_neuronx_cc_hook()

    partition_name = nc.partition_id_tensor.name if nc.partition_id_tensor else None
    in_names, out_names, out_avals = [], [], []
    for alloc in nc.m.functions[0].allocations:
        if not isinstance(alloc, mybir.MemoryLocationSet):
            continue
        name = alloc.memorylocations[0].name
        if alloc.kind == "ExternalInput":
            if name != partition_name:
                in_names.append(name)
        elif alloc.kind == "ExternalOutput":
            out_names.append(name)
            out_avals.append(
                jax.core.ShapedArray(tuple(alloc.tensor_shape), mybir.dt.np(alloc.dtype))
            )

    bind_names = list(in_names) + ([partition_name] if partition_name else [])

    def _body(*args):
        operands = list(args)
        if partition_name is not None:
            operands.append(bass2jax.partition_id_tensor())
        outs = bass2jax._bass_exec_p.bind(
            *operands,
            out_avals=tuple(out_avals),
            in_names=tuple(bind_names),
            out_names=tuple(out_names),
            lowering_input_output_aliases=(),
            sim_require_finite=False,
            sim_require_nnan=False,
            nc=nc,
        )
        return tuple(outs)

    devices = jax.devices()[:n_cores]
    mesh = Mesh(np.asarray(devices), ("core",))
    fn = jax.jit(
        shard_map(
            _body,
            mesh=mesh,
            in_specs=(PartitionSpec("core"),) * len(in_names),
            out_specs=(PartitionSpec("core"),) * len(out_names),
            check_rep=False,
        )
    )
    return fn, in_names, mesh


def _run_variant(nc, in_maps, n_iters=10):
    """Compile nc, stage inputs on device once, return (min_wall_s, outputs)."""
    import time as _time
    import jax

    fn, in_names, mesh = _make_runner(nc)
    concat = [
        np.concatenate([np.asarray(in_maps[c][n]) for c in range(N_CORES)], axis=0)
        for n in in_names
    ]
    dev_in = [jax.device_put(a) for a in concat]
    outs = fn(*dev_in)
    jax.block_until_ready(outs)
    best = float("inf")
    for _ in range(n_iters):
        t0 = _time.perf_counter()
        outs = fn(*dev_in)
        jax.block_until_ready(outs)
        best = min(best, _time.perf_counter() - t0)
    return best, outs


def time_kernel(reps=513, n_iters=6, inputs=None):
    """Estimate single-iteration HW time by differencing a reps-looped
    program against the reps=1 program. Returns nanoseconds."""
    if inputs is None:
        rng = np.random.default_rng(0)
        s = 1.0 / np.sqrt(D)
        inputs = dict(
            x=rng.standard_normal((B, S, D)).astype(np.float32),
            Wq=(rng.standard_normal((D, D)) * s).astype(np.float32),
            Wk=(rng.standard_normal((D, D)) * s).astype(np.float32),
            Wv=(rng.standard_normal((D, D)) * s).astype(np.float32),
            bq=(rng.standard_normal(D) * 0.02).astype(np.float32),
            bk=(rng.standard_normal(D) * 0.02).astype(np.float32),
            bv=(rng.standard_normal(D) * 0.02).astype(np.float32),
        )
    in_maps = build_in_maps(**inputs)
    # mock_cc: the pair AllGather is replaced by equal-volume local DMAs in
    # BOTH timing variants (a collective inside a long For_i loop desyncs
    # the device); the real kernel() path keeps the true collectives.
    t1, o1 = _run_variant(build_program(reps=1, mock_cc=True), in_maps, n_iters)
    tR, oR = _run_variant(build_program(reps=reps, mock_cc=True), in_maps, n_iters)
    per_iter = (tR - t1) / (reps - 1)
    print(f"t1={t1*1e3:.2f}ms  t{reps}={tR*1e3:.2f}ms  per-iter={per_iter*1e6:.1f}us")
    return per_iter * 1e9



# revision 11
# speedup vs baseline: 1.3316x; 1.3316x over previous
"""Causal single-head attention (B=4, S=2048, D=1024) on 8 Trainium2 cores.

Sharding: core c -> (batch b = c//2, half h = c%2). Each core owns 1024
query rows: the 8 seq-blocks of 128 rows with global block index
g = 2*j + h (j = 0..7), which balances causal work between the two
halves. Each core projects K/V for all 2048 keys of its batch and Q for
its own rows, runs block-causal attention, and writes out^T [D, 1024].
The host transposes/scatters the per-core outputs back together.

Precision: hybrid fp8(e4m3)/bf16. All projections, scores and AV run as
fp8 DoubleRow matmuls (2 contraction sub-tiles per pass, 2x PE
throughput). fp8 noise does not average out for the earliest causal
rows (row r attends only r+1 keys), so the first global 256 rows
(= local block 0 on every core, global block h) are recomputed through
a small bf16 path: bf16 projections of block-0 rows against full bf16
weights, bf16 scores/AV against the (<=256) visible keys, and the
block-0 output columns are written from that path instead.

On-chip layout (fp8 path; everything contracts over the partition dim,
DR = DoubleRow pairs two adjacent dim-1 sub-tiles per matmul):
  Q^T[d, q]  = sum_e W8q[e, d] * x8T[e, q]   (DR over e-pairs)
  K^T[d, s]  likewise; V[s, d] with x8T chunk stationary
  S^T[k, q]  = sum_d K8T[d, k] * Q8T[d, q]   (DR over d-pairs)
  E^T[k, q]  = mask * exp(S^T / 32)          (ACT exp, DVE mask-mult, fp8 out)
  den[q]     = sum_k E^T[k, q]               (ones-matmul on PE)
  O^T[d, q]  = sum_k V8[k, d] * E^T[k, q] / den[q]  (DR over key-chunk pairs)

Each core projects Q/K/V only for its own 1024 rows; K^T and V are
exchanged with the pair partner via a 2-core AllGather (DRAM bounce) in
fp8, plus one small bf16 AllGather carrying block-0 K^T/V for the early
path. Gathered order (even half then odd half) is identical on both
cores, so the SPMD program is uniform and causality is pure mask data
computed on the host per core.
"""

import os
import sys

sys.path.insert(0, "/opt/trn_rl_repo")

# The jit wrapper used for timing carries fp8 parameters; TRN2's HLO
# verifier wants them declared as the 240-max e4m3 variant. All payloads
# here are <= 240, where the two encodings are bit-identical.
_F8_FLAG = "--experimental-unsafe-fp8e4m3fn-as-fp8e4m3"
if _F8_FLAG not in os.environ.get("NEURON_CC_FLAGS", ""):
    os.environ["NEURON_CC_FLAGS"] = (
        os.environ.get("NEURON_CC_FLAGS", "") + " " + _F8_FLAG
    ).strip()

import numpy as np
import ml_dtypes

import concourse.bass as bass
import concourse.tile as tile
import concourse.mybir as mybir
from concourse.bass_utils import run_bass_kernel_spmd
from concourse.vector_clock import ScopedClock, VectorClock

BF16 = mybir.dt.bfloat16
F8 = mybir.dt.float8e4
F32 = mybir.dt.float32
AF = mybir.ActivationFunctionType
DR = mybir.MatmulPerfMode.DoubleRow

B, S, D = 4, 2048, 1024
SL = 1024  # local query rows per core
N_CORES = 8
W8SCALE = 16.0  # fp8 weights stored as 16*W so values sit in e4m3 normal range


def _patch_ldw_opt():
    """Re-enable walrus's redundant-LDWEIGHTS elimination (off by default in
    this runtime). Our matmuls come in same-stationary pairs, so eliding the
    duplicate weight load saves about half the LDWEIGHTS traffic."""
    import os
    import concourse.bass_utils as bu

    if getattr(bu, "_ldw_opt_patched", False):
        return
    bu._ldw_opt_patched = True
    orig = bu.run_command

    def wrapped(argv, **kwargs):
        if os.environ.get("KERNEL_LDW_OPT") == "1":
            argv = [
                a.replace("--enable-ldw-opt=false", "--enable-ldw-opt=true")
                for a in argv
            ]
        return orig(argv, **kwargs)

    bu.run_command = wrapped


def _patch_tile_runtime():
    """Two local-runtime fixes:
    1. This walrus build rejects >1 semaphore wait on a Drain instruction;
       split the TileContext exit drain into one drain per awaited proc.
    2. Raise the stale 192KB/partition SBUF cap to 208KB (cayman usable).
    """
    import concourse.tile_utils as tile_utils

    tile_utils.max_sbuf_usage = 208 * 1024

    def _drain_and_barrier(self, tick_clock, wait_clock):
        g = tick_clock.global_clock
        n = len(g)
        for j in range(n):
            if g[j] > 0:
                sub = VectorClock([g[i] if i == j else 0 for i in range(n)])
                drain_inst = self.nc.sync.drain()
                wait_clock.add_sem_waits(drain_inst.ins, ScopedClock({None: sub}))
        self.nc.all_engine_barrier()
        assert self.sems is not None
        popped = self.nc._tile_sem_poison_stack.pop()
        assert popped is self._sem_poison
        self.nc.clear_and_free_semaphores(list(self.sems.allocated().values()))
        self.nc.all_engine_barrier()

    tile.TileContext._drain_and_barrier = _drain_and_barrier


def _local_blocks(h):
    """Global 128-row block indices owned by half h, in local order."""
    return [2 * j + h for j in range(8)]


def _key_perm_blocks(h=None):
    """Key 128-blocks in on-chip (gathered) order: even half then odd half.
    Identical on both cores of a pair (AllGather slot order)."""
    return _local_blocks(0) + _local_blocks(1)


def _chunk_list(t):
    """Key-chunk slots (into the permuted order) computed for q-tile t."""
    return list(range(0, 4 * t + 4)) + list(range(8, 12 + 4 * t))


def _masked_chunks(t):
    """(slot, mask_j) pairs needing a mask for q-tile t: own and partner
    chunks 4t..4t+3 (mask_j 0..3 own, 4..7 partner)."""
    out = []
    for j in range(4):
        out.append((4 * t + j, j))  # own chunk 4t+j
        out.append((8 + 4 * t + j, 4 + j))  # partner chunk 4t+j
    return out


def build_masks(h):
    """mask[t, j, k, q] in {0,1}: for q-tile t, mask_j as in _masked_chunks."""
    qg = np.empty(SL, dtype=np.int64)  # local q index -> global row
    for j in range(8):
        qg[j * 128 : (j + 1) * 128] = 128 * (2 * j + h) + np.arange(128)
    kperm = _key_perm_blocks(h)
    mask = np.zeros((2, 8, 128, 512), dtype=np.float32)
    for t in range(2):
        qs = qg[t * 512 : (t + 1) * 512]  # [512] global q rows
        for slot, j in _masked_chunks(t):
            kb = kperm[slot]  # global key block
            ks = 128 * kb + np.arange(128)  # [128] global key rows
            mask[t, j] = (qs[None, :] >= ks[:, None]).astype(np.float32)
    return mask.astype(ml_dtypes.bfloat16)


def build_maske(h):
    """Early-path mask [2, 128, 128]: global q rows 128h..128h+127 vs key
    blocks 0 and 1 (gathered early slots)."""
    qs = 128 * h + np.arange(128)
    m = np.zeros((2, 128, 128), dtype=np.float32)
    for kb in range(2):
        ks = 128 * kb + np.arange(128)
        m[kb] = (ks[:, None] <= qs[None, :]).astype(np.float32)
    return m.astype(ml_dtypes.bfloat16)


def _split_waits(nc, limit=1, dma_limit=1):
    """This walrus build accepts at most `limit` semaphore waits per
    engine instruction (and `dma_limit` per DMA descriptor). Hoist extra
    waits onto same-engine NoOps inserted immediately before the
    offending instruction (same program point, so ordering semantics are
    unchanged; for DMAs the wait moves from trigger-time to issue-time on
    the issuing sequencer, which only strengthens ordering)."""
    n_split = 0
    for f in nc.m.functions:
        for bb in f.blocks:
            new = []
            for inst in bb.instructions:
                si = inst.sync_info
                waits = list(si.on_wait) if si and si.on_wait else []
                lim = dma_limit if type(inst).__name__ == "InstDMACopy" else limit
                if len(waits) > lim:
                    extra, keep = waits[:-lim], waits[-lim:]
                    for j in range(0, len(extra), limit):
                        nop = mybir.InstNoOp(
                            name=f"{inst.name}-wsplit{j}", ins=[], outs=[]
                        )
                        nop.engine = inst.engine
                        nop.sync_info = mybir.SyncInfo(
                            on_wait=extra[j : j + limit], on_update=[]
                        )
                        new.append(nop)
                        n_split += 1
                    si.on_wait = keep
                new.append(inst)
            bb.instructions[:] = new
    return n_split


def _dedup_ldw(nc):
    """With the PE stream pinned to emission order, paired matmuls share
    their stationary operand back-to-back; convert the second (identical)
    InstLdweights into a NoOp carrying the same sync_info."""
    n = 0
    for f in nc.m.functions:
        for bb in f.blocks:
            prev_sig = None
            for idx, inst in enumerate(bb.instructions):
                if inst.engine != mybir.EngineType.PE:
                    continue
                nm = type(inst).__name__
                if nm == "InstLdweights":
                    sig = str(inst.ins[0])
                    if sig == prev_sig:
                        nop = mybir.InstNoOp(
                            name=f"{inst.name}-ldwdup", ins=[], outs=[]
                        )
                        nop.engine = inst.engine
                        nop.sync_info = inst.sync_info
                        bb.instructions[idx] = nop
                        n += 1
                    prev_sig = sig
                elif nm != "InstMatmult":
                    prev_sig = None
    return n


def build_program(split_waits=True, reps=1, mock_cc=False, pin_pe=True):
    _patch_tile_runtime()
    _patch_ldw_opt()
    nc = bass.Bass("TRN2", target_bir_lowering=False, debug=False)

    xT8d = nc.dram_tensor("xT8", [D, SL], F8, kind="ExternalInput").ap()
    xTed = nc.dram_tensor("xTe", [D, 128], BF16, kind="ExternalInput").ap()
    w8q = nc.dram_tensor("w8q", [D, D], F8, kind="ExternalInput").ap()
    w8k = nc.dram_tensor("w8k", [D, D], F8, kind="ExternalInput").ap()
    w8v = nc.dram_tensor("w8v", [D, D], F8, kind="ExternalInput").ap()
    wbq = nc.dram_tensor("wbq", [D, D], BF16, kind="ExternalInput").ap()
    wbk = nc.dram_tensor("wbk", [D, D], BF16, kind="ExternalInput").ap()
    wbv = nc.dram_tensor("wbv", [D, D], BF16, kind="ExternalInput").ap()
    bqd = nc.dram_tensor("bq", [D], F32, kind="ExternalInput").ap()
    bkd = nc.dram_tensor("bk", [D], F32, kind="ExternalInput").ap()
    bvb = nc.dram_tensor("bvb", [128, D], BF16, kind="ExternalInput").ap()
    maskd = nc.dram_tensor("mask", [2, 8, 128, 512], BF16, kind="ExternalInput").ap()
    masked_e = nc.dram_tensor("maske", [2, 128, 128], BF16, kind="ExternalInput").ap()
    outT = nc.dram_tensor("outT", [D, SL], F32, kind="ExternalOutput").ap()

    with tile.TileContext(nc) as tc:
        with (
            tc.tile_pool(name="persist", bufs=1) as persist,
            tc.tile_pool(name="w8pool", bufs=2) as w8pool,
            tc.tile_pool(name="wbpool", bufs=2) as wbpool,
            tc.tile_pool(name="xpool", bufs=2) as xpool,
            tc.tile_pool(name="xg", bufs=1) as xg,
            tc.tile_pool(name="mk", bufs=4) as mkp,
            tc.tile_pool(name="scratch", bufs=4) as scratch,
            tc.tile_pool(name="outp", bufs=3) as outp,
            tc.tile_pool(name="bc", bufs=2) as bcp,
            tc.tile_pool(name="dram", bufs=1, space="DRAM") as drp,
            tc.tile_pool(name="ps", bufs=8, space="PSUM") as psp,
        ):
            def emit_body():
                # Pin the PE stream to emission order so same-stationary
                # matmul pairs stay adjacent (enables the LDWEIGHTS dedup
                # post-pass); deps are order-only on a single engine.
                last_mm = [None]

                def MM(*a, **kw):
                    inst = nc.tensor.matmul(*a, **kw)
                    if pin_pe:
                        if last_mm[0] is not None:
                            bass._add_dep_helper(
                                inst.ins, last_mm[0].ins, False, "pe-order-pin"
                            )
                        last_mm[0] = inst
                    return inst

                # ---- persistent SBUF tiles ----
                QT8 = persist.tile([128, 8, SL], F8)  # [dp, dc, q]
                KT8 = persist.tile([128, 8, S], F8)  # [dp, dc, k]
                V8 = persist.tile([128, 16, D], F8)  # [kp, kc, d]
                ET0 = persist.tile([128, 8, 512], F8)  # q-tile 0
                ET1 = persist.tile([128, 16, 512], F8)  # q-tile 1
                ET = [ET0, ET1]
                QTe = persist.tile([128, 8, 128], BF16)  # early q (bf16)
                # early keys: slot-major [dp, slot g, dc*128 + k] (matches
                # the packed exchange payload flat layout exactly)
                KTe = persist.tile([128, 2, 1024], BF16)
                Ve = persist.tile([128, 2, D], BF16)
                eE = persist.tile([128, 2, 128], BF16)
                maske_t = persist.tile([128, 2, 128], BF16)
                bq_t = persist.tile([128, 8], F32)
                bk_t = persist.tile([128, 8], F32)
                bv_t = persist.tile([128, D], BF16)
                ones8 = persist.tile([128, 1], F8)
                onesb = persist.tile([128, 1], BF16)
                ones_row = persist.tile([1, 128], F32)

                nc.vector.memset(ones8, 1.0)
                nc.vector.memset(onesb, 1.0)
                nc.vector.memset(ones_row, 1.0)
                nc.sync.dma_start(out=bq_t, in_=bqd.rearrange("(c p) -> p c", p=128))
                nc.sync.dma_start(out=bk_t, in_=bkd.rearrange("(c p) -> p c", p=128))
                nc.sync.dma_start(out=bv_t, in_=bvb)
                nc.sync.dma_start(
                    out=maske_t, in_=masked_e.rearrange("s p q -> p s q")
                )

                def load_w(dram, dt, tag, split=False):
                    wt = (w8pool if dt == F8 else wbpool).tile(
                        [128, 8, D], dt, tag=tag
                    )
                    src_ap = dram.rearrange("(c p) d -> p c d", p=128)
                    if split:
                        # per-d-chunk DMAs: the first matmul group only
                        # waits on its own slice, not the whole tensor
                        for dc in range(8):
                            nc.sync.dma_start(
                                out=wt[:, :, dc * 128 : (dc + 1) * 128],
                                in_=src_ap[:, :, dc * 128 : (dc + 1) * 128],
                            )
                    else:
                        nc.sync.dma_start(out=wt, in_=src_ap)
                    return wt

                xt8 = xpool.tile([128, 8, SL], F8, tag="x8")
                nc.sync.dma_start(
                    out=xt8, in_=xT8d.rearrange("(c p) s -> p c s", p=128)
                )
                xte = xpool.tile([128, 8, 128], BF16, tag="xe")
                nc.sync.dma_start(
                    out=xte, in_=xTed.rearrange("(c p) s -> p c s", p=128)
                )

                pair_groups = [[0, 1], [2, 3], [4, 5], [6, 7]]

                def exchange(own, ib_shape, dt, tag):
                    ib = drp.tile(ib_shape, dt, tag="ib" + tag, name="ib" + tag)
                    ob = drp.tile([2] + ib_shape, dt, tag="ob" + tag, name="ob" + tag)
                    if mock_cc:
                        nc.sync.dma_start(out=ib, in_=own)
                        for g in range(2):
                            nc.sync.dma_start(out=ob[g], in_=ib)
                    else:
                        nc.gpsimd.dma_start(out=ib, in_=own)
                        nc.gpsimd.collective_compute(
                            "AllGather",
                            mybir.AluOpType.bypass,
                            replica_groups=pair_groups,
                            ins=[ib[:]],
                            outs=[ob[:]],
                        )
                    return ob

                # ---- Phase K (fp8 DR proj of own keys), kick the exchange
                wk_t = load_w(w8k, F8, "w8", split=True)
                KT8own = xg.tile([128, 8, SL], F8, tag="kown", name="KT8own")
                for dc in range(8):
                    ps0 = psp.tile([128, 512], F32, tag="ps")
                    ps1 = psp.tile([128, 512], F32, tag="ps")
                    for ep in range(4):
                        w = wk_t[:, 2 * ep : 2 * ep + 2, dc * 128 : (dc + 1) * 128]
                        MM(ps0, w, xt8[:, 2 * ep : 2 * ep + 2, 0:512],
                           start=(ep == 0), stop=(ep == 3), perf_mode=DR)
                        MM(ps1, w, xt8[:, 2 * ep : 2 * ep + 2, 512:1024],
                           start=(ep == 0), stop=(ep == 3), perf_mode=DR)
                    for sh, ps in ((0, ps0), (1, ps1)):
                        nc.scalar.activation(
                            out=KT8own[:, dc, sh * 512 : (sh + 1) * 512],
                            in_=ps,
                            func=AF.Identity,
                            bias=bk_t[:, dc : dc + 1],
                            scale=1.0 / W8SCALE,
                        )
                obK = exchange(KT8own, [128, 8, SL], F8, "K")
                for g in range(2):
                    nc.sync.dma_start(
                        out=KT8[:, :, g * SL : (g + 1) * SL], in_=obK[g]
                    )

                # ---- Phase Q (fp8 DR)
                wq_t = load_w(w8q, F8, "w8")
                for dc in range(8):
                    ps0 = psp.tile([128, 512], F32, tag="ps")
                    ps1 = psp.tile([128, 512], F32, tag="ps")
                    for ep in range(4):
                        w = wq_t[:, 2 * ep : 2 * ep + 2, dc * 128 : (dc + 1) * 128]
                        MM(ps0, w, xt8[:, 2 * ep : 2 * ep + 2, 0:512],
                           start=(ep == 0), stop=(ep == 3), perf_mode=DR)
                        MM(ps1, w, xt8[:, 2 * ep : 2 * ep + 2, 512:1024],
                           start=(ep == 0), stop=(ep == 3), perf_mode=DR)
                    for sh, ps in ((0, ps0), (1, ps1)):
                        nc.scalar.activation(
                            out=QT8[:, dc, sh * 512 : (sh + 1) * 512],
                            in_=ps,
                            func=AF.Identity,
                            bias=bq_t[:, dc : dc + 1],
                            scale=1.0 / W8SCALE,
                        )

                # ---- Phase V (fp8 DR, x^T chunk stationary, row-major V)
                wv_t = load_w(w8v, F8, "w8")
                V8own = xg.tile([128, 8, D], F8, tag="vown", name="V8own")
                for kc in range(8):
                    ps0 = psp.tile([128, 512], F32, tag="ps")
                    ps1 = psp.tile([128, 512], F32, tag="ps")
                    for ep in range(4):
                        xl = xt8[:, 2 * ep : 2 * ep + 2, kc * 128 : (kc + 1) * 128]
                        MM(ps0, xl, wv_t[:, 2 * ep : 2 * ep + 2, 0:512],
                           start=(ep == 0), stop=(ep == 3), perf_mode=DR)
                        MM(ps1, xl, wv_t[:, 2 * ep : 2 * ep + 2, 512:1024],
                           start=(ep == 0), stop=(ep == 3), perf_mode=DR)
                    for dh, ps in ((0, ps0), (1, ps1)):
                        vs = scratch.tile([128, 512], BF16, tag="vs")
                        nc.scalar.activation(
                            out=vs, in_=ps, func=AF.Identity, scale=1.0 / W8SCALE
                        )
                        nc.vector.tensor_tensor(
                            out=V8own[:, kc, dh * 512 : (dh + 1) * 512],
                            in0=vs,
                            in1=bv_t[:, dh * 512 : (dh + 1) * 512],
                            op=mybir.AluOpType.add,
                        )
                obV = exchange(V8own, [128, 8, D], F8, "V")
                for g in range(2):
                    nc.sync.dma_start(out=V8[:, 8 * g : 8 * g + 8, :], in_=obV[g])

                # ---- Early bf16 projections of local block 0 (128 rows).
                # K^T/V go into the pack tile for the small bf16 exchange;
                # Q^T stays local.
                pack = xg.tile([128, 2048], BF16, tag="pack", name="pack")
                wbk_t = load_w(wbk, BF16, "wb", split=True)
                for dc in range(8):
                    pse_t = psp.tile([128, 128], F32, tag="ps")
                    for ec in range(8):
                        MM(pse_t, wbk_t[:, ec, dc * 128 : (dc + 1) * 128],
                           xte[:, ec, :], start=(ec == 0), stop=(ec == 7))
                    nc.scalar.activation(
                        out=pack[:, dc * 128 : (dc + 1) * 128],
                        in_=pse_t,
                        func=AF.Identity,
                        bias=bk_t[:, dc : dc + 1],
                        scale=1.0,
                    )
                wbq_t = load_w(wbq, BF16, "wb", split=True)
                for dc in range(8):
                    pse_t = psp.tile([128, 128], F32, tag="ps")
                    for ec in range(8):
                        MM(pse_t, wbq_t[:, ec, dc * 128 : (dc + 1) * 128],
                           xte[:, ec, :], start=(ec == 0), stop=(ec == 7))
                    nc.scalar.activation(
                        out=QTe[:, dc, :],
                        in_=pse_t,
                        func=AF.Identity,
                        bias=bq_t[:, dc : dc + 1],
                        scale=1.0,
                    )
                # V early: x chunk stationary, row-major [128 rows, D]
                wbv_t = load_w(wbv, BF16, "wb")
                psv0 = psp.tile([128, 512], F32, tag="ps")
                psv1 = psp.tile([128, 512], F32, tag="ps")
                for ec in range(8):
                    xl = xte[:, ec, :]
                    MM(psv0, xl, wbv_t[:, ec, 0:512],
                       start=(ec == 0), stop=(ec == 7))
                    MM(psv1, xl, wbv_t[:, ec, 512:1024],
                       start=(ec == 0), stop=(ec == 7))
                for dh, ps in ((0, psv0), (1, psv1)):
                    nc.vector.tensor_tensor(
                        out=pack[:, 1024 + dh * 512 : 1024 + (dh + 1) * 512],
                        in0=ps,
                        in1=bv_t[:, dh * 512 : (dh + 1) * 512],
                        op=mybir.AluOpType.add,
                    )
                obE = exchange(pack, [128, 2048], BF16, "E")
                for g in range(2):
                    nc.sync.dma_start(out=KTe[:, g, :], in_=obE[g][:, 0:1024])
                    nc.sync.dma_start(out=Ve[:, g, :], in_=obE[g][:, 1024:2048])

                # ---- attention scores (fp8 DR): both q-tiles per key chunk
                # so each K^T chunk LDWEIGHTS feeds two matmuls
                set0 = set(_chunk_list(0))
                c1 = _chunk_list(1)
                md0 = dict(_masked_chunks(0))
                md1 = dict(_masked_chunks(1))
                n0 = len(_chunk_list(0))
                dp0 = psp.tile([1, 512], F32, tag="ps")
                dp1 = psp.tile([1, 512], F32, tag="ps")

                # fp8e4 on this HW tops out at 240, and raw scores reach
                # ~z=5.6 sigma (exp ~ 264). Shift the fp8-path exponent by a
                # global constant: E' = exp(s/32 - ESHIFT). Numerator and
                # denominator of the softmax scale identically, so the ratio
                # is exact; it just buys ~e^ESHIFT of fp8 headroom.
                ESHIFT = float(np.log(8.0))
                eshift_t = persist.tile([128, 1], F32)
                nc.vector.memset(eshift_t, -ESHIFT)

                def do_exp(t, i, c, sp, md):
                    if c in md:
                        mt = mkp.tile([128, 512], BF16, tag="mk")
                        nc.sync.dma_start(out=mt, in_=maskd[t, md[c]])
                        ex = scratch.tile([128, 512], BF16, tag="ex")
                        nc.scalar.activation(
                            out=ex, in_=sp, func=AF.Exp, scale=1.0 / 32.0,
                            bias=eshift_t,
                        )
                        nc.vector.tensor_tensor(
                            out=ET[t][:, i, :],
                            in0=ex,
                            in1=mt,
                            op=mybir.AluOpType.mult,
                        )
                    else:
                        nc.scalar.activation(
                            out=ET[t][:, i, :], in_=sp, func=AF.Exp,
                            scale=1.0 / 32.0, bias=eshift_t,
                        )

                i0 = 0
                for i1, c in enumerate(c1):
                    sp1 = psp.tile([128, 512], F32, tag="ps")
                    sp0 = (
                        psp.tile([128, 512], F32, tag="ps", name=f"sp0_{i1}")
                        if c in set0
                        else None
                    )
                    for dp_ in range(4):
                        kt = KT8[:, 2 * dp_ : 2 * dp_ + 2, c * 128 : (c + 1) * 128]
                        MM(sp1, kt, QT8[:, 2 * dp_ : 2 * dp_ + 2, 512:1024],
                           start=(dp_ == 0), stop=(dp_ == 3), perf_mode=DR)
                        if sp0 is not None:
                            MM(sp0, kt, QT8[:, 2 * dp_ : 2 * dp_ + 2, 0:512],
                               start=(dp_ == 0), stop=(dp_ == 3), perf_mode=DR)
                    do_exp(1, i1, c, sp1, md1)
                    if sp0 is not None:
                        do_exp(0, i0, c, sp0, md0)
                        i0 += 1

                # denominators: ones-column stationary (1-column LDW ~ free)
                for t, dp, n in ((0, dp0, n0), (1, dp1, len(c1))):
                    for i in range(n):
                        MM(dp, ones8, ET[t][:, i, :], start=(i == 0), stop=(i == n - 1))

                # ---- early path: scores over the two early key blocks
                for kb in range(2):
                    spE = psp.tile([128, 128], F32, tag="ps")
                    for dc in range(8):
                        MM(spE, KTe[:, kb, dc * 128 : (dc + 1) * 128],
                           QTe[:, dc, :], start=(dc == 0), stop=(dc == 7))
                    ex = scratch.tile([128, 128], BF16, tag="exe")
                    nc.scalar.activation(out=ex, in_=spE, func=AF.Exp, scale=1.0 / 32.0)
                    nc.vector.tensor_tensor(
                        out=eE[:, kb, :],
                        in0=ex,
                        in1=maske_t[:, kb, :],
                        op=mybir.AluOpType.mult,
                    )
                dpE = psp.tile([1, 128], F32, tag="ps")
                for kb in range(2):
                    MM(dpE, onesb, eE[:, kb, :], start=(kb == 0), stop=(kb == 1))

                # reciprocal + partition-broadcast via rank-1 PE outer product
                rbs = []
                for dp in (dp0, dp1):
                    rec = scratch.tile([1, 512], F32, tag="rec")
                    nc.vector.reciprocal(out=rec, in_=dp)
                    rbp = psp.tile([128, 512], F32, tag="ps")
                    MM(rbp, ones_row, rec, start=True, stop=True)
                    rb = bcp.tile([128, 512], F32, tag="rb")
                    nc.vector.tensor_copy(rb, rbp)
                    rbs.append(rb)
                recE = scratch.tile([1, 128], F32, tag="rece")
                nc.vector.reciprocal(out=recE, in_=dpE)
                rbpE = psp.tile([128, 128], F32, tag="ps")
                MM(rbpE, ones_row, recE, start=True, stop=True)
                rbE = bcp.tile([128, 128], F32, tag="rbe")
                nc.vector.tensor_copy(rbE, rbpE)

                # ---- early AV + block-0 output columns
                for dc in range(8):
                    avE = psp.tile([128, 128], F32, tag="ps")
                    for kb in range(2):
                        MM(avE, Ve[:, kb, dc * 128 : (dc + 1) * 128],
                           eE[:, kb, :], start=(kb == 0), stop=(kb == 1))
                    otE = outp.tile([128, 128], F32, tag="ote")
                    nc.vector.tensor_tensor(
                        out=otE, in0=avE, in1=rbE, op=mybir.AluOpType.mult
                    )
                    nc.sync.dma_start(
                        out=outT[dc * 128 : (dc + 1) * 128, 0:128], in_=otE
                    )

                # ---- AV (fp8 DR): both q-tiles per (d-half, key chunk-pair);
                # V chunk stationary feeds two matmuls; 8 PSUM accumulators
                # ET0 slot pairs: chunk pairs (0,1),(2,3),(8,9),(10,11)
                e0pair_of = {0: 0, 1: 1, 4: 2, 5: 3}  # cp -> ET0 slot pair idx
                for dh in range(2):
                    av1 = [
                        psp.tile([128, 512], F32, tag="ps", name=f"av1_{dh}_{j}")
                        for j in range(4)
                    ]
                    av0 = [
                        psp.tile([128, 512], F32, tag="ps", name=f"av0_{dh}_{j}")
                        for j in range(4)
                    ]
                    for cp in range(8):  # gathered key-chunk pair (2cp, 2cp+1)
                        in0 = cp in e0pair_of
                        for j in range(4):
                            dc = 4 * dh + j
                            vt = V8[:, 2 * cp : 2 * cp + 2, dc * 128 : (dc + 1) * 128]
                            MM(av1[j], vt, ET1[:, 2 * cp : 2 * cp + 2, :],
                               start=(cp == 0), stop=(cp == 7), perf_mode=DR)
                            if in0:
                                ep = e0pair_of[cp]
                                MM(av0[j], vt, ET0[:, 2 * ep : 2 * ep + 2, :],
                                   start=(ep == 0), stop=(cp == 5), perf_mode=DR)
                    for t, avs in ((1, av1), (0, av0)):
                        for j in range(4):
                            dc = 4 * dh + j
                            if t == 0:
                                # block-0 columns come from the early path
                                ot = outp.tile([128, 384], F32, tag="ot0")
                                nc.vector.tensor_tensor(
                                    out=ot,
                                    in0=avs[j][:, 128:512],
                                    in1=rbs[0][:, 128:512],
                                    op=mybir.AluOpType.mult,
                                )
                                nc.sync.dma_start(
                                    out=outT[dc * 128 : (dc + 1) * 128, 128:512],
                                    in_=ot,
                                )
                            else:
                                ot = outp.tile([128, 512], F32, tag="ot")
                                nc.vector.tensor_tensor(
                                    out=ot,
                                    in0=avs[j],
                                    in1=rbs[1],
                                    op=mybir.AluOpType.mult,
                                )
                                nc.sync.dma_start(
                                    out=outT[
                                        dc * 128 : (dc + 1) * 128, 512:1024
                                    ],
                                    in_=ot,
                                )

            if reps == 1:
                emit_body()
            else:
                with tc.For_i(0, reps, 1):
                    emit_body()

    if pin_pe:
        _dedup_ldw(nc)
    if split_waits:
        _split_waits(nc)
    return nc


_prog_cache = {}


def build_in_maps(x, Wq, bq, Wk, bk, Wv, bv):
    x = np.asarray(x, dtype=np.float32)
    Wq = np.asarray(Wq, dtype=np.float32)
    Wk = np.asarray(Wk, dtype=np.float32)
    Wv = np.asarray(Wv, dtype=np.float32)
    bq_np = np.asarray(bq, dtype=np.float32)
    bk_np = np.asarray(bk, dtype=np.float32)
    bv_np = np.asarray(bv, dtype=np.float32)

    E4 = ml_dtypes.float8_e4m3fn
    w8q_h = np.ascontiguousarray(Wq.T * W8SCALE).astype(E4)
    w8k_h = np.ascontiguousarray(Wk.T * W8SCALE).astype(E4)
    w8v_h = np.ascontiguousarray(Wv.T * W8SCALE).astype(E4)
    wbq_h = np.ascontiguousarray(Wq.T).astype(ml_dtypes.bfloat16)
    wbk_h = np.ascontiguousarray(Wk.T).astype(ml_dtypes.bfloat16)
    wbv_h = np.ascontiguousarray(Wv.T).astype(ml_dtypes.bfloat16)
    bvb = np.broadcast_to(bv_np.astype(ml_dtypes.bfloat16), (128, D)).copy()
    masks = [build_masks(h) for h in range(2)]
    maskes = [build_maske(h) for h in range(2)]

    in_maps = []
    for c in range(N_CORES):
        b, h = divmod(c, 2)
        own = np.concatenate(
            [128 * g + np.arange(128) for g in _local_blocks(h)]
        )
        xT = np.ascontiguousarray(x[b].T[:, own])
        in_maps.append(
            {
                "xT8": xT.astype(E4),
                "xTe": xT[:, 0:128].astype(ml_dtypes.bfloat16),
                "w8q": w8q_h,
                "w8k": w8k_h,
                "w8v": w8v_h,
                "wbq": wbq_h,
                "wbk": wbk_h,
                "wbv": wbv_h,
                "bq": bq_np,
                "bk": bk_np,
                "bvb": bvb,
                "mask": masks[h],
                "maske": maskes[h],
            }
        )
    return in_maps


def unshard(outTs):
    """outTs: list of 8 per-core outT arrays [D, SL] -> full [B, S, D]."""
    out = np.empty((B, S, D), dtype=np.float32)
    for c in range(N_CORES):
        b, h = divmod(c, 2)
        rows = np.concatenate([128 * g + np.arange(128) for g in _local_blocks(h)])
        out[b, rows, :] = outTs[c].T
    return out


def kernel(x, Wq, bq, Wk, bk, Wv, bv):
    if "nc" not in _prog_cache:
        _prog_cache["nc"] = build_program()
    nc = _prog_cache["nc"]
    in_maps = build_in_maps(x, Wq, bq, Wk, bk, Wv, bv)
    try:
        res = run_bass_kernel_spmd(nc, in_maps, list(range(N_CORES)))
    except Exception:
        # one retry absorbs transient device/mesh hiccups
        import time as _t

        _t.sleep(2.0)
        res = run_bass_kernel_spmd(nc, in_maps, list(range(N_CORES)))
    return unshard([res.results[c]["outT"] for c in range(N_CORES)])


# ---------------------------------------------------------------------------
# Timing: run a reps-times-looped program variant through a no-donation jit
# wrapper with device-resident inputs, and difference wall-clock floors.


def _make_runner(nc, n_cores=N_CORES):
    import jax
    from jax.experimental.shard_map import shard_map
    from jax.sharding import Mesh, PartitionSpec
    from concourse import bass2jax

    bass2jax.install_neuronx_cc_hook()

    partition_name = nc.partition_id_tensor.name if nc.partition_id_tensor else None
    in_names, out_names, out_avals = [], [], []
    for alloc in nc.m.functions[0].allocations:
        if not isinstance(alloc, mybir.MemoryLocationSet):
            continue
        name = alloc.memorylocations[0].name
        if alloc.kind == "ExternalInput":
            if name != partition_name:
                in_names.append(name)
        elif alloc.kind == "ExternalOutput":
            out_names.append(name)
            out_avals.append(
                jax.core.ShapedArray(tuple(alloc.tensor_shape), mybir.dt.np(alloc.dtype))
            )

    bind_names = list(in_names) + ([partition_name] if partition_name else [])

    def _body(*args):
        operands = list(args)
        if partition_name is not None:
            operands.append(bass2jax.partition_id_tensor())
        outs = bass2jax._bass_exec_p.bind(
            *operands,
            out_avals=tuple(out_avals),
            in_names=tuple(bind_names),
            out_names=tuple(out_names),
            lowering_input_output_aliases=(),
            sim_require_finite=False,
            sim_require_nnan=False,
            nc=nc,
        )
        return tuple(outs)

    devices = jax.devices()[:n_cores]
    mesh = Mesh(np.asarray(devices), ("core",))
    fn = jax.jit(
        shard_map(
            _body,
            mesh=mesh,
            in_specs=(PartitionSpec("core"),) * len(in_names),
            out_specs=(PartitionSpec("core"),) * len(out_names),
            check_rep=False,
        )
    )
    return fn, in_names, mesh


def _run_variant(nc, in_maps, n_iters=10):
    """Compile nc, stage inputs on device once, return (min_wall_s, outputs)."""
    import time as _time
    import jax

    fn, in_names, mesh = _make_runner(nc)
    concat = []
    for n in in_names:
        a = np.concatenate(
            [np.asarray(in_maps[c][n]) for c in range(N_CORES)], axis=0
        )
        if a.dtype == ml_dtypes.float8_e4m3fn:
            # XLA-on-TRN2 rejects f8e4m3fn at module boundaries; ship the
            # bytes as u8 (the NEFF binds buffers by position and size).
            a = a.view(np.uint8)
        concat.append(a)
    dev_in = [jax.device_put(a) for a in concat]
    outs = fn(*dev_in)
    jax.block_until_ready(outs)
    best = float("inf")
    for _ in range(n_iters):
        t0 = _time.perf_counter()
        outs = fn(*dev_in)
        jax.block_until_ready(outs)
        best = min(best, _time.perf_counter() - t0)
    return best, outs


def time_kernel(reps=513, n_iters=6, inputs=None):
    """Estimate single-iteration HW time by differencing a reps-looped
    program against the reps=1 program. Returns nanoseconds."""
    if inputs is None:
        rng = np.random.default_rng(0)
        s = 1.0 / np.sqrt(D)
        inputs = dict(
            x=rng.standard_normal((B, S, D)).astype(np.float32),
            Wq=(rng.standard_normal((D, D)) * s).astype(np.float32),
            Wk=(rng.standard_normal((D, D)) * s).astype(np.float32),
            Wv=(rng.standard_normal((D, D)) * s).astype(np.float32),
            bq=(rng.standard_normal(D) * 0.02).astype(np.float32),
            bk=(rng.standard_normal(D) * 0.02).astype(np.float32),
            bv=(rng.standard_normal(D) * 0.02).astype(np.float32),
        )
    in_maps = build_in_maps(**inputs)
    # mock_cc: the pair AllGather is replaced by equal-volume local DMAs in
    # BOTH timing variants (a collective inside a long For_i loop desyncs
    # the device); the real kernel() path keeps the true collectives.
    t1, o1 = _run_variant(build_program(reps=1, mock_cc=True), in_maps, n_iters)
    tR, oR = _run_variant(build_program(reps=reps, mock_cc=True), in_maps, n_iters)
    per_iter = (tR - t1) / (reps - 1)
    print(f"t1={t1*1e3:.2f}ms  t{reps}={tR*1e3:.2f}ms  per-iter={per_iter*1e6:.1f}us")
    return per_iter * 1e9
